# revision 1
# baseline (speedup 1.0000x reference)
"""Trainium2 Bass kernel for nn_EnhancedPatchOptimizedModel.

Strategy: pure data-parallel over batch (128 -> 16 per core x 8 cores).
Per-core compute is a straight-line Tile program:
  - activations kept "feature-major": [D on partitions (16 chunks of 128), rows on free]
  - all big linears: stationary = weight chunk [128 din, 128 dout], moving = activations
    (c-outer loop, 16 packed PSUM accumulators, weights streamed as quarter-matrices)
  - weights pre-cast to bf16 on host; fp32 PSUM accumulation
  - O(N^2) relation MLP: fused DVE add + ACT relu + PE dot with w_rel2
  - row-major detours (LayerNorm, ctx/att@v/agg contractions over rows) via
    DMA-transpose (bf16 xbar) + PE-transpose for the 16-row remainders
"""
import sys
sys.path.insert(0, "/opt/trn_rl_repo")

import math
import numpy as np
import ml_dtypes

import concourse.bass as bass
import concourse.tile as tile
from concourse import bacc, mybir
from concourse.bass_utils import run_bass_kernel_spmd

F32 = mybir.dt.float32
BF16 = mybir.dt.bfloat16

NCORES = 8
B, N, D = 128, 9, 2048
BL = B // NCORES          # 16 samples per core
R = BL * N                # 144 rows per core
CH = D // 128             # 16 feature chunks
H, DK = 4, 512
HID, NCLS = 1024, 101
EPS = 1e-5
RT = [(0, 128), (128, 16)]   # row tiles for row-major world


def _bd(x):  # host cast to bf16
    return np.ascontiguousarray(x).astype(ml_dtypes.bfloat16)


def _colmaj(v):  # [D] -> [128, D//128] fp32 (chunk c in column c)
    return np.ascontiguousarray(v.reshape(-1, 128).T).astype(np.float32)


# ----------------------------------------------------------------------------
# device program
# ----------------------------------------------------------------------------

class _Done(Exception):
    pass


def build_nc():
    import os
    STAGE = float(os.environ.get("BASS_STAGE", "99"))
    nc = bacc.Bacc("TRN2", target_bir_lowering=False, debug=False,
                   enable_asserts=False, num_devices=NCORES)

    def din(name, shape, dt=F32):
        return nc.dram_tensor(name, shape, dt, kind="ExternalInput")

    x0T_d = din("x0T", [D, R])
    wnames = ["wda1", "wda2", "wr1a", "wr1b", "wctx", "wq", "wk", "wv", "wo"]
    wd = {n: din(n, [D, D], BF16) for n in wnames}
    wc1_d = din("wc1", [D, HID], BF16)
    wc2_d = din("wc2", [HID, NCLS], BF16)
    w2_d = din("w2col", [128, CH], BF16)          # w_rel2 chunks as columns
    w2p_d = din("w2pad", [128, CH * 128], BF16)   # chunk c at col c*128, rest 0
    bias_names = ["bda1", "bda2", "brel1", "bctx", "lng", "lnb", "bq", "bk", "bv"]
    bd_ = {n: din(n + "T", [128, CH]) for n in bias_names}
    sbn_d = din("sbnT", [128, HID // 128])
    bcls_d = din("bclsT", [128, HID // 128])
    bc2_d = din("bc2c", [NCLS, 1])
    brel2_d = din("brel2", [1, 1])

    out_d = nc.dram_tensor("outT", [NCLS, BL], F32, kind="ExternalOutput")
    sscr_d = nc.dram_tensor("sscr", [BL * N * N], F32, kind="Internal")
    gscr_d = nc.dram_tensor("gscr", [R], BF16, kind="Internal")
    bdscr_d = nc.dram_tensor("bdscr", [N, BL, N], BF16, kind="Internal")
    ascr_d = nc.dram_tensor("ascr", [H, N, BL, N], BF16, kind="Internal")

    # inline constants
    mask9_np = (1.0 - np.eye(N, dtype=np.float32))
    mask9_d = nc.inline_tensor(mask9_np, name="mask9")
    id128_d = nc.inline_tensor(np.eye(128, dtype=ml_dtypes.bfloat16), name="id128")
    ones9_d = nc.inline_tensor(np.ones((N, 1), dtype=ml_dtypes.bfloat16), name="ones9")

    with tile.TileContext(nc) as tc:
        import contextlib
        ctx = contextlib.ExitStack()
        with ctx:
            pw = ctx.enter_context(tc.tile_pool(name="pw", bufs=3))
            p1 = ctx.enter_context(tc.tile_pool(name="p1", bufs=1))
            p2 = ctx.enter_context(tc.tile_pool(name="p2", bufs=2))
            p4 = ctx.enter_context(tc.tile_pool(name="p4", bufs=4))
            p5 = ctx.enter_context(tc.tile_pool(name="p5", bufs=5))
            qp = ctx.enter_context(tc.tile_pool(name="qp", bufs=8, space="PSUM"))

            # ---------------- constants to SBUF ----------------
            def cload(dram, shape, dt=F32, tag=None):
                t = p1.tile(shape, dt, tag=tag, name=tag)
                nc.sync.dma_start(t[:], dram.ap())
                return t

            SKIPC = int(os.environ.get("BASS_SKIP_CONSTS", "0"))
            bias = {} if SKIPC else {n: cload(bd_[n], [128, CH], tag="b_" + n) for n in bias_names}
            w2sb = cload(w2_d, [128, CH], BF16, tag="w2sb")
            w2pad = cload(w2p_d, [128, CH * 128], BF16, tag="w2pad")
            sbn_sb = None if SKIPC else cload(sbn_d, [128, HID // 128], tag="sbn")
            bcls_sb = None if SKIPC else cload(bcls_d, [128, HID // 128], tag="bcls")
            bc2_sb = None if SKIPC else cload(bc2_d, [NCLS, 1], tag="bc2")
            mask9 = None if SKIPC else cload(mask9_d, [N, N], tag="mask9")
            id128 = None if SKIPC else cload(id128_d, [128, 128], BF16, tag="id128")
            ones9 = None if SKIPC else cload(ones9_d, [N, 1], BF16, tag="ones9")
            brel2 = p1.tile([N, 1], F32, tag="brel2")
            nc.sync.dma_start(brel2[:], brel2_d.ap().to_broadcast((N, 1)))
            epst = p1.tile([128, 1], F32, tag="epst")
            nc.vector.memset(epst[:], EPS)

            # ---------------- input activations ----------------
            X0 = p2.tile([128, CH * R], F32, tag="fmf32")
            nc.sync.dma_start(
                X0[:].rearrange("p (c r) -> p c r", c=CH),
                x0T_d.ap().rearrange("(c p) r -> p c r", p=128))
            X0b = p5.tile([128, CH * R], BF16, tag="fmb16")
            nc.vector.tensor_copy(X0b[:], X0[:])

            # ---------------- weight streaming ----------------
            def wquarters(wdram, dout, nq=4):
                """Yield per-quarter SBUF tiles viewed as [128, 4, dout]."""
                tiles = []
                rows_per_q = D // nq
                for q in range(nq):
                    t = pw.tile([128, (rows_per_q // 128) * dout], BF16, tag="W", name=f"wq{q}")
                    nc.sync.dma_start(
                        t[:].rearrange("p (cl d) -> p cl d", d=dout),
                        wdram.ap()[q * rows_per_q:(q + 1) * rows_per_q, :]
                        .rearrange("(cl p) d -> p cl d", p=128))
                    tiles.append(t)
                return tiles

            def fm_linear(wdram, rhs_sb, dout, epil):
                """Feature-major linear: out_T[dout_chunk m] = sum_c W[c,m].T @ rhs[c]."""
                qt = wquarters(wdram, dout)
                mb = dout // 128
                ngr = (mb + 2) // 3
                accs = [qp.tile([128, 3 * R], F32, tag="pb", name=f"acc{_g}") for _g in range(ngr)]
                for c in range(CH):
                    w_q = qt[c // 4][:].rearrange("p (cl d) -> p cl d", d=dout)
                    for m in range(mb):
                        g, sl = divmod(m, 3)
                        glast = min(3 * g + 2, mb - 1) - 3 * g
                        nc.tensor.matmul(
                            accs[g][:, sl * R:(sl + 1) * R],
                            w_q[:, c % 4, m * 128:(m + 1) * 128],
                            rhs_sb[:, c * R:(c + 1) * R],
                            start=(c == 0 and sl == 0),
                            stop=(c == CH - 1 and sl == glast))
                for m in range(mb):
                    g, sl = divmod(m, 3)
                    epil(m, accs[g][:, sl * R:(sl + 1) * R])

            AF = mybir.ActivationFunctionType
            OP = mybir.AluOpType

            try:
                def gate(s):
                    if STAGE < s:
                        zt = p1.tile([NCLS, BL], F32, tag="zdum", name="zdum")
                        nc.vector.memset(zt[:], 0.0)
                        nc.sync.dma_start(out_d.ap(), zt[:])
                        raise _Done

                # ---------------- S1: domain adaptation ----------------
                SKIP1 = int(os.environ.get("BASS_SKIP_S1", "0"))
                T1b = p5.tile([128, CH * R], BF16, tag="fmb16")
                if SKIP1:
                    nc.vector.tensor_copy(T1b[:], X0b[:])
                else:
                    fm_linear(wd["wda1"], X0b, D, lambda m, ps: nc.scalar.activation(
                        T1b[:, m * R:(m + 1) * R], ps, AF.Relu, bias=bias["bda1"][:, m:m + 1]))

                X1 = p2.tile([128, CH * R], F32, tag="fmf32")
                X1b = p5.tile([128, CH * R], BF16, tag="fmb16")

                def epi_da2(m, ps):
                    nc.scalar.activation(X1[:, m * R:(m + 1) * R], ps, AF.Identity,
                                         bias=bias["bda2"][:, m:m + 1])
                    nc.vector.tensor_copy(X1b[:, m * R:(m + 1) * R],
                                          X1[:, m * R:(m + 1) * R])
                if SKIP1:
                    nc.vector.tensor_copy(X1[:], T1b[:])
                    nc.vector.tensor_copy(X1b[:], T1b[:])
                else:
                    fm_linear(wd["wda2"], T1b, D, epi_da2)

                gate(1)
                # ---------------- S2: relation a/b sides ----------------
                if int(os.environ.get("BASS_SKIP_S2", "0")):
                    Ab = Bb = X1b
                else:
                    Ab = p5.tile([128, CH * R], BF16, tag="fmb16")
                    fm_linear(wd["wr1a"], X1b, D,
                              lambda m, ps: nc.vector.tensor_copy(Ab[:, m * R:(m + 1) * R], ps))
                    Bb = p5.tile([128, CH * R], BF16, tag="fmb16")
                    fm_linear(wd["wr1b"], X1b, D, lambda m, ps: nc.vector.tensor_scalar_add(
                        Bb[:, m * R:(m + 1) * R], ps, bias["brel1"][:, m:m + 1]))

                # x1 row-major (bf16) for ctx contraction
                x1row = [p4.tile([128, D], BF16, tag="rowa", name="x1rowa"),
                         p4.tile([16, D], BF16, tag="rowb", name="x1rowb")]

                def pe_t(dst_ap, src_ap, ident):
                    """dst = src.T via PE transpose (bf16), PSUM bounce + DVE copy."""
                    pt = qp.tile([src_ap.shape[1], src_ap.shape[0]], BF16, tag="pb",
                                 name="pt")
                    nc.tensor.matmul(pt[:], src_ap, ident, is_transpose=True)
                    nc.vector.tensor_copy(dst_ap, pt[:])

                def to_row(srcT, dst):
                    """Feature-major [128, CH*R] bf16 -> row-major tiles [128,D]+[16,D]."""
                    for c in range(CH):
                        pe_t(dst[0][:128, c * 128:(c + 1) * 128],
                             srcT[:, c * R:c * R + 128], id128[:])
                        pe_t(dst[1][0:16, c * 128:(c + 1) * 128],
                             srcT[:, c * R + 128:c * R + 144], id128[:])

                def to_feat(rows, dstT):
                    """Row-major [128,D]+[16,D] bf16 -> feature-major [128, CH*R] bf16."""
                    for c in range(CH):
                        pe_t(dstT[:, c * R:c * R + 128],
                             rows[0][:128, c * 128:(c + 1) * 128], id128[:])
                        pe_t(dstT[:, c * R + 128:c * R + 144],
                             rows[1][0:16, c * 128:(c + 1) * 128], id128[0:16, 0:16])

                if not int(os.environ.get("BASS_SKIP_TOROW", "0")):
                    to_row(X1b, x1row)

                gate(2)
                # ---------------- S3: pairwise scores ----------------
                psc = [qp.tile([128, 512], F32, tag="pb", name=f"psc{_s}") for _s in range(3)]
                HMODE = os.environ.get("BASS_H_MODE", "full")
                for c in range(CH):
                    h1 = p2.tile([128, BL * N * N], BF16, tag="H1", name="h1")
                    a_v = (Ab[:, c * R:(c + 1) * R]
                           .rearrange("p (b i) -> p b i", i=N)[:, :, :, None]
                           .broadcast_to((128, BL, N, N)))
                    b_v = (Bb[:, c * R:(c + 1) * R]
                           .rearrange("p (b j) -> p b j", j=N)[:, :, None, :]
                           .broadcast_to((128, BL, N, N)))
                    if HMODE == "nott":
                        nc.vector.tensor_copy(h1[:], Ab[:, 0:BL * N * N])
                    else:
                        nc.vector.tensor_add(
                            h1[:].rearrange("p (b i j) -> p b i j", i=N, j=N), a_v, b_v)
                    h2 = p2.tile([128, BL * N * N], BF16, tag="H2", name="h2")
                    if HMODE in ("full", "nott", "norelu"):
                        if HMODE == "norelu":
                            nc.vector.tensor_copy(h2[:], h1[:])
                        else:
                            nc.scalar.activation(h2[:], h1[:], AF.Relu)
                        for g in range(9):
                            s, sl = divmod(g, 3)
                            nc.tensor.matmul(psc[s][:, sl * R:(sl + 1) * R],
                                             w2pad[:, c * 128:(c + 1) * 128],
                                             h2[:, g * R:(g + 1) * R],
                                             start=(c == 0 and sl == 0),
                                             stop=(c == CH - 1 and sl == 2))
                if HMODE == "ttonly":
                    gate(2.1)
                gate(2.2)
                ssb = p1.tile([1, BL * N * N], F32, tag="ssb")
                for g in range(9):
                    s, sl = divmod(g, 3)
                    nc.vector.tensor_copy(ssb[:, g * R:(g + 1) * R],
                                          psc[s][0:1, sl * R:(sl + 1) * R])
                nc.sync.dma_start(sscr_d.ap(), ssb[0:1, :].rearrange("p f -> (p f)"))

                # scores -> compact [9, (16 b, 9 j)] layout (partition = query i)
                Ssb = p2.tile([N, BL * N], F32, tag="srel")
                nc.sync.dma_start(
                    Ssb[:].rearrange("p (b j) -> p b j", j=N),
                    sscr_d.ap().rearrange("(b i j) -> i b j", b=BL, i=N, j=N))

                gate(2.4)
                # mask diagonal, add b_rel2:  V2 = (S + brel2) * mask
                V2 = p2.tile([N, BL * N], F32, tag="srel")
                vw = V2[:].rearrange("p (b j) -> p b j", j=N)
                m_v = mask9[:, None, :].broadcast_to((N, BL, N))
                nc.vector.scalar_tensor_tensor(
                    vw, Ssb[:].rearrange("p (b j) -> p b j", j=N), brel2[:], m_v,
                    OP.add, OP.mult)

                # softmax over j
                EA = p2.tile([N, BL * N], F32, tag="srel")
                ew = EA[:].rearrange("p (b j) -> p b j", j=N)
                nc.scalar.activation(ew, vw, AF.Exp)
                ssum = p1.tile([N, BL], F32, tag="ssum")
                nc.vector.reduce_sum(ssum[:], ew, axis=mybir.AxisListType.X)
                srcp = p1.tile([N, BL], F32, tag="srcp")
                nc.vector.reciprocal(srcp[:], ssum[:])
                relwb = p2.tile([N, BL * N], BF16, tag="srelb")
                nc.vector.tensor_mul(
                    relwb[:].rearrange("p (b j) -> p b j", j=N),
                    ew, srcp[:, :, None].broadcast_to((N, BL, N)))

                gate(2.6)
                def build_bd(src_ibj_view, scr_ap, name):
                    """src [i part, b, j] view -> DRAM [i, b, j] -> block-diag
                    BD[(b,j), (b,i)] as two tiles [128, R] + [16, R] bf16."""
                    nc.sync.dma_start(scr_ap, src_ibj_view)
                    bda = p1.tile([128, R], BF16, tag=name + "a", name=name + "a")
                    bdb = p1.tile([16, R], BF16, tag=name + "b", name=name + "b")
                    nc.vector.memset(bda[:], 0.0)
                    nc.vector.memset(bdb[:], 0.0)
                    for b in range(BL):
                        # [j, i] = transpose of scr[:, b, :]; singleton last dim keeps
                        # the DMA's final dim contiguous
                        blk = scr_ap[:, b, :].rearrange("i j -> j i")[:, :, None]
                        if b <= 13:
                            nc.sync.dma_start(
                                bda[b * N:b * N + N, b * N:b * N + N][:, :, None], blk)
                        elif b == 14:
                            nc.sync.dma_start(bda[126:128, 126:135][:, :, None], blk[0:2])
                            nc.sync.dma_start(bdb[0:7, 126:135][:, :, None], blk[2:9])
                        else:
                            nc.sync.dma_start(bdb[7:16, 135:144][:, :, None], blk)
                    return bda, bdb

                bdr_a, bdr_b = build_bd(
                    relwb[:].rearrange("p (b j) -> p b j", j=N), bdscr_d.ap(), "bdr")

                gate(3)
                # ---------------- ctx = relw @ x1 (row-major out) ----------------
                DSL = [(s * 512, 512) for s in range(4)]

                def bd_mm_evac(bda, bdb, rows, slices, dst):
                    """dst[(b,i), s0:s0+sw] = sum_(b,j) BD.T @ rows, via bank psums."""
                    for (c0, csz, dt_) in ((0, 128, dst[0]), (128, 16, dst[1])):
                        for s0, sw_ in slices:
                            ps = qp.tile([csz, sw_], F32, tag="pb", name="pbd")
                            nc.tensor.matmul(ps[:], bda[:, c0:c0 + csz],
                                             rows[0][:, s0:s0 + sw_],
                                             start=True, stop=False)
                            nc.tensor.matmul(ps[:], bdb[:, c0:c0 + csz],
                                             rows[1][0:16, s0:s0 + sw_],
                                             start=False, stop=True)
                            nc.scalar.activation(dt_[0:csz, s0:s0 + sw_], ps[:], AF.Copy)

                ctxrow = [p4.tile([128, D], BF16, tag="rowa", name="ctxrowa"),
                          p4.tile([16, D], BF16, tag="rowb", name="ctxrowb")]
                bd_mm_evac(bdr_a, bdr_b, x1row, DSL, ctxrow)
                ctxT = p5.tile([128, CH * R], BF16, tag="fmb16")
                to_feat(ctxrow, ctxT)

                gate(4)
                # ---------------- wctx linear + LayerNorm ----------------
                ctx2T = p5.tile([128, CH * R], BF16, tag="fmb16")
                fm_linear(wd["wctx"], ctxT, D, lambda m, ps: nc.scalar.activation(
                    ctx2T[:, m * R:(m + 1) * R], ps, AF.Identity,
                    bias=bias["bctx"][:, m:m + 1]))

                c2row = [p4.tile([128, D], BF16, tag="rowa", name="c2rowa"),
                         p4.tile([16, D], BF16, tag="rowb", name="vrowb")]
                to_row(ctx2T, c2row)

                ctxnrow = [p4.tile([128, D], BF16, tag="rowa", name="cnrowa"),
                           p4.tile([16, D], BF16, tag="rowb", name="cnrowb")]
                for t, (lo, sz) in enumerate(RT):
                    src = c2row[t][0:sz, :]
                    nmu = p1.tile([128, 1], F32, tag=f"nmu{t}", name=f"nmu{t}")
                    nc.vector.tensor_reduce(nmu[0:sz], src, axis=mybir.AxisListType.X,
                                            op=OP.add, negate=True)
                    nmus = p1.tile([128, 1], F32, tag=f"nmus{t}", name=f"nmus{t}")
                    nc.vector.tensor_scalar_mul(nmus[0:sz], nmu[0:sz], 1.0 / D)
                    tt = p4.tile([128, D], BF16, tag="lnt", name=f"lnt{t}")
                    nc.scalar.activation(tt[0:sz, :], src, AF.Identity, bias=nmus[0:sz])
                    dum = p4.tile([128, D], BF16, tag="lndum", name=f"lndum{t}")
                    var = p1.tile([128, 1], F32, tag=f"var{t}", name=f"var{t}")
                    nc.vector.tensor_tensor_reduce(
                        dum[0:sz, :], tt[0:sz, :], tt[0:sz, :], 1.0 / D, 0.0,
                        OP.mult, OP.add, accum_out=var[0:sz])
                    sd = p1.tile([128, 1], F32, tag=f"sd{t}", name=f"sd{t}")
                    nc.scalar.activation(sd[0:sz], var[0:sz], AF.Sqrt, bias=epst[0:sz])
                    rstd = p1.tile([128, 1], F32, tag=f"rstd{t}", name=f"rstd{t}")
                    nc.vector.reciprocal(rstd[0:sz], sd[0:sz])
                    nc.vector.tensor_scalar_mul(ctxnrow[t][0:sz, :], tt[0:sz, :],
                                                rstd[0:sz])

                ctxnT = p5.tile([128, CH * R], BF16, tag="fmb16")
                to_feat(ctxnrow, ctxnT)

                # residual + affine: Xe = X1 + g*ctxn + lnb
                Xeb = p5.tile([128, CH * R], BF16, tag="fmb16")
                for c in range(CH):
                    sl = slice(c * R, (c + 1) * R)
                    t1 = p2.tile([128, R], F32, tag="resid", name=f"resid{c}")
                    nc.vector.scalar_tensor_tensor(
                        t1[:], ctxnT[:, sl], bias["lng"][:, c:c + 1],
                        bias["lnb"][:, c:c + 1].to_broadcast((128, R)), OP.mult, OP.add)
                    nc.vector.tensor_add(Xeb[:, sl], t1[:], X1[:, sl])

                gate(5)
                # ---------------- S5: attention ----------------
                QT = p5.tile([128, CH * R], BF16, tag="fmb16")
                fm_linear(wd["wq"], Xeb, D, lambda m, ps: nc.scalar.activation(
                    QT[:, m * R:(m + 1) * R], ps, AF.Identity, bias=bias["bq"][:, m:m + 1]))
                KT = p5.tile([128, CH * R], BF16, tag="fmb16")
                fm_linear(wd["wk"], Xeb, D, lambda m, ps: nc.scalar.activation(
                    KT[:, m * R:(m + 1) * R], ps, AF.Identity, bias=bias["bk"][:, m:m + 1]))
                VT = p5.tile([128, CH * R], BF16, tag="fmb16")
                fm_linear(wd["wv"], Xeb, D, lambda m, ps: nc.scalar.activation(
                    VT[:, m * R:(m + 1) * R], ps, AF.Identity, bias=bias["bv"][:, m:m + 1]))

                vrow = [p4.tile([128, D], BF16, tag="rowa", name="vrowa"),
                        p4.tile([16, D], BF16, tag="rowb", name="vrowb")]
                to_row(VT, vrow)

                # attention scores per sample: att[i, (b,h,j)]
                patt = [qp.tile([N, 8 * H * N], F32, tag="pb", name=f"patt{_p}") for _p in range(2)]
                for b in range(BL):
                    pa = patt[b // 8]
                    for c in range(CH):
                        h = c // (CH // H)
                        dst = pa[:, (b % 8) * H * N + h * N:(b % 8) * H * N + h * N + N]
                        nc.tensor.matmul(dst, QT[:, c * R + b * N:c * R + b * N + N],
                                         KT[:, c * R + b * N:c * R + b * N + N],
                                         start=(b % 8 == 0 and c == 0),
                                         stop=(b % 8 == 7 and c == CH - 1))

                gate(6)
                # att scores -> compact [9 i, (16 b, 4 h, 9 j)] + scale
                SM = p2.tile([N, BL * H * N], F32, tag="sat")
                for b in range(BL):
                    nc.scalar.activation(
                        SM[:, b * H * N:(b + 1) * H * N],
                        patt[b // 8][:, (b % 8) * H * N:(b % 8 + 1) * H * N],
                        AF.Copy, scale=1.0 / math.sqrt(DK))

                aw = SM[:].rearrange("p (g j) -> p g j", j=N)
                EA2 = p2.tile([N, BL * H * N], F32, tag="sat")
                e2w = EA2[:].rearrange("p (g j) -> p g j", j=N)
                nc.scalar.activation(e2w, aw, AF.Exp)
                asum = p1.tile([N, BL * H], F32, tag="asum")
                nc.vector.reduce_sum(asum[:], e2w, axis=mybir.AxisListType.X)
                arcp = p1.tile([N, BL * H], F32, tag="arcp")
                nc.vector.reciprocal(arcp[:], asum[:])
                attb = p2.tile([N, BL * H * N], BF16, tag="satb")
                nc.vector.tensor_mul(
                    attb[:].rearrange("p (g j) -> p g j", j=N),
                    e2w, arcp[:, :, None].broadcast_to((N, BL * H, N)))

                gate(7)
                # att @ v per head via block-diag
                oavrow = [p4.tile([128, D], BF16, tag="rowa", name="oavrowa"),
                          p4.tile([16, D], BF16, tag="rowb", name="oavrowb")]
                for h in range(H):
                    bda_h, bdb_h = build_bd(
                        attb[:].rearrange("p (b h j) -> p h b j", h=H, j=N)[:, h],
                        ascr_d.ap()[h], f"bda{h}")
                    bd_mm_evac(bda_h, bdb_h, vrow, [(h * DK, DK)], oavrow)
                OAVT = p5.tile([128, CH * R], BF16, tag="fmb16")
                to_feat(oavrow, OAVT)

                gate(8)
                # out = oav @ wo  (row-major out via moving weights)
                woq = wquarters(wd["wo"], D)
                por = {}
                for t, (lo, sz) in enumerate(RT):
                    for s0, sw_ in DSL:
                        por[(t, s0)] = qp.tile([sz, sw_], F32, tag="pb",
                                               name=f"por{t}_{s0}")
                for c in range(CH):
                    w_q = woq[c // 4][:].rearrange("p (cl d) -> p cl d", d=D)
                    for t, (lo, sz) in enumerate(RT):
                        for s0, sw_ in DSL:
                            nc.tensor.matmul(por[(t, s0)][:],
                                             OAVT[:, c * R + lo:c * R + lo + sz],
                                             w_q[:, c % 4, s0:s0 + sw_],
                                             start=(c == 0), stop=(c == CH - 1))
                outrow = [p4.tile([128, D], BF16, tag="rowa", name="orowa"),
                          p4.tile([16, D], BF16, tag="rowb", name="orowb")]
                for t, (lo, sz) in enumerate(RT):
                    for s0, sw_ in DSL:
                        nc.vector.tensor_copy(outrow[t][0:sz, s0:s0 + sw_],
                                              por[(t, s0)][:])

                gate(9)
                # ---------------- gw = softmax(mean att) ; agg ----------------
                pimp = qp.tile([1, R], F32, tag="pb", name="pimp")
                for h in range(H):
                    nc.tensor.matmul(
                        pimp[:], ones9[:],
                        attb[:].rearrange("p (b h j) -> p h b j", h=H, j=N)[:, h],
                        start=(h == 0), stop=(h == H - 1))
                egw = p1.tile([1, R], F32, tag="egw")
                nc.scalar.activation(egw[:].rearrange("p (b j) -> p b j", j=N),
                                     pimp[:].rearrange("p (b j) -> p b j", j=N),
                                     AF.Exp, scale=1.0 / (H * N))
                gsum = p1.tile([1, BL], F32, tag="gsum")
                nc.vector.reduce_sum(gsum[:], egw[:].rearrange("p (b j) -> p b j", j=N),
                                     axis=mybir.AxisListType.X)
                grcp = p1.tile([1, BL], F32, tag="grcp")
                nc.vector.reciprocal(grcp[:], gsum[:])
                gwb = p1.tile([1, R], BF16, tag="gwb")
                nc.vector.tensor_mul(gwb[:].rearrange("p (b j) -> p b j", j=N),
                                     egw[:].rearrange("p (b j) -> p b j", j=N),
                                     grcp[:, :, None].broadcast_to((1, BL, N)))
                nc.sync.dma_start(gscr_d.ap(), gwb[0:1, :].rearrange("p f -> (p f)"))
                gcol1 = p1.tile([128, 1], BF16, tag="gcol1")
                nc.sync.dma_start(gcol1[:], gscr_d.ap()[0:128].rearrange("(p f) -> p f", f=1))
                gcol2 = p1.tile([16, 1], BF16, tag="gcol2")
                nc.sync.dma_start(gcol2[:], gscr_d.ap()[128:144].rearrange("(p f) -> p f", f=1))
                BD1 = p1.tile([128, BL], BF16, tag="BD1")
                BD2 = p1.tile([16, BL], BF16, tag="BD2")
                nc.vector.memset(BD1[:], 0.0)
                nc.vector.memset(BD2[:], 0.0)
                for b in range(14):
                    nc.sync.dma_start(BD1[b * N:min(b * N + N, 128), b:b + 1],
                                      gcol1[b * N:min(b * N + N, 128), :])
                nc.sync.dma_start(BD1[126:128, 14:15], gcol1[126:128, :])
                nc.sync.dma_start(BD2[0:7, 14:15], gcol2[0:7, :])
                nc.sync.dma_start(BD2[7:16, 15:16], gcol2[7:16, :])

                aggb = p1.tile([BL, D], BF16, tag="aggb")
                for s0, sw_ in DSL:
                    pagg = qp.tile([BL, sw_], F32, tag="pb", name=f"pagg{s0}")
                    nc.tensor.matmul(pagg[:], BD1[:], outrow[0][:, s0:s0 + sw_],
                                     start=True, stop=False)
                    nc.tensor.matmul(pagg[:], BD2[:], outrow[1][0:16, s0:s0 + sw_],
                                     start=False, stop=True)
                    nc.vector.tensor_copy(aggb[:, s0:s0 + sw_], pagg[:])

                aggT = p1.tile([128, CH * BL], BF16, tag="aggT")
                for c in range(CH):
                    pe_t(aggT[:, c * BL:(c + 1) * BL],
                         aggb[:, c * 128:(c + 1) * 128], id128[0:16, 0:16])

                gate(10)
                # ---------------- classifier ----------------
                wc1q = wquarters(wc1_d, HID)
                pc1 = qp.tile([128, (HID // 128) * BL], F32, tag="pb", name="pc1")
                for c in range(CH):
                    w_q = wc1q[c // 4][:].rearrange("p (cl d) -> p cl d", d=HID)
                    for m in range(HID // 128):
                        nc.tensor.matmul(pc1[:, m * BL:(m + 1) * BL],
                                         w_q[:, c % 4, m * 128:(m + 1) * 128],
                                         aggT[:, c * BL:(c + 1) * BL],
                                         start=(c == 0 and m == 0),
                                         stop=(c == CH - 1 and m == HID // 128 - 1))
                Y1b = p1.tile([128, (HID // 128) * BL], BF16, tag="Y1b")
                for m in range(HID // 128):
                    nc.scalar.activation(Y1b[:, m * BL:(m + 1) * BL],
                                         pc1[:, m * BL:(m + 1) * BL], AF.Relu,
                                         bias=bcls_sb[:, m:m + 1], scale=sbn_sb[:, m:m + 1])

                wc2sb = p1.tile([128, (HID // 128) * NCLS], BF16, tag="wc2sb")
                nc.sync.dma_start(
                    wc2sb[:].rearrange("p (hc n) -> p hc n", n=NCLS),
                    wc2_d.ap().rearrange("(hc p) n -> p hc n", p=128))
                pout = qp.tile([NCLS, BL], F32, tag="pb", name="pout")
                for hc in range(HID // 128):
                    nc.tensor.matmul(pout[:], wc2sb[:, hc * NCLS:(hc + 1) * NCLS],
                                     Y1b[:, hc * BL:(hc + 1) * BL],
                                     start=(hc == 0), stop=(hc == HID // 128 - 1))
                OUTsb = p1.tile([NCLS, BL], F32, tag="OUTsb")
                nc.vector.tensor_scalar_add(OUTsb[:], pout[:], bc2_sb[:])
                nc.sync.dma_start(out_d.ap(), OUTsb[:])
            except _Done:
                pass

    nc.compile()
    return nc


# ----------------------------------------------------------------------------
# host side
# ----------------------------------------------------------------------------

def host_inputs(inputs):
    """Build the shared (weight) part of the per-core input map."""
    f32 = np.float32
    wda1, wda2 = inputs["wda1"], inputs["wda2"]
    w_rel1 = inputs["w_rel1"]
    m = {
        "wda1": _bd(wda1), "wda2": _bd(wda2),
        "wr1a": _bd(w_rel1[:D]), "wr1b": _bd(w_rel1[D:]),
        "wctx": _bd(inputs["w_ctx"]), "wq": _bd(inputs["wq"]),
        "wk": _bd(inputs["wk"]), "wv": _bd(inputs["wv"]), "wo": _bd(inputs["wo"]),
        "wc1": _bd(inputs["wc1"]), "wc2": _bd(inputs["wc2"]),
        "w2col": _bd(_colmaj(inputs["w_rel2"][:, 0])),
        "w2pad": _bd(np.concatenate(
            [np.concatenate([_colmaj(inputs["w_rel2"][:, 0])[:, c:c + 1],
                             np.zeros((128, 127), np.float32)], axis=1)
             for c in range(CH)], axis=1)),
        "bda1T": _colmaj(inputs["bda1"]), "bda2T": _colmaj(inputs["bda2"]),
        "brel1T": _colmaj(inputs["b_rel1"]), "bctxT": _colmaj(inputs["b_ctx"]),
        "lngT": _colmaj(inputs["ln_g"]), "lnbT": _colmaj(inputs["ln_b"]),
        "bqT": _colmaj(inputs["bq"]), "bkT": _colmaj(inputs["bk"]),
        "bvT": _colmaj(inputs["bv"]),
        "brel2": np.asarray(inputs["b_rel2"], f32).reshape(1, 1),
        "bc2c": np.asarray(inputs["bc2"], f32).reshape(NCLS, 1),
    }
    s = np.asarray(inputs["bn_g"], f32) / np.sqrt(np.asarray(inputs["bn_v"], f32) + EPS)
    bo_w = np.asarray(inputs["bo"], f32) @ np.asarray(inputs["wc1"], f32)
    bias2 = (np.asarray(inputs["bc1"], f32) + bo_w
             - np.asarray(inputs["bn_m"], f32)) * s + np.asarray(inputs["bn_b"], f32)
    m["sbnT"] = _colmaj(s)
    m["bclsT"] = _colmaj(bias2)
    return m


def core_input(inputs, shared, core):
    pf = np.asarray(inputs["patch_features"], np.float32)
    shard = pf[core * BL:(core + 1) * BL].reshape(R, D)
    m = dict(shared)
    m["x0T"] = np.ascontiguousarray(shard.T)
    return m


_NC_CACHE = {}


def get_nc():
    if "nc" not in _NC_CACHE:
        _NC_CACHE["nc"] = build_nc()
    return _NC_CACHE["nc"]




def _reference_numpy(inputs):
    """Exact fp32 fallback (matches the reference computation)."""
    f32 = np.float32
    x = np.asarray(inputs["patch_features"], f32)
    Bf, Nf, Df = x.shape
    relu = lambda v: np.maximum(v, 0)
    x = relu(x @ np.asarray(inputs["wda1"], f32) + inputs["bda1"]) \
        @ np.asarray(inputs["wda2"], f32) + inputs["bda2"]
    a = x @ np.asarray(inputs["w_rel1"], f32)[:Df]
    b = x @ np.asarray(inputs["w_rel1"], f32)[Df:]
    h = relu(a[:, :, None, :] + b[:, None, :, :] + inputs["b_rel1"])
    scores = (h @ np.asarray(inputs["w_rel2"], f32))[..., 0] + inputs["b_rel2"][0]
    scores = scores * (1.0 - np.eye(Nf, dtype=f32))
    e = np.exp(scores - scores.max(axis=2, keepdims=True))
    relw = e / e.sum(axis=2, keepdims=True)
    ctx = np.einsum('bij,bjd->bid', relw, x)
    ctx = ctx @ np.asarray(inputs["w_ctx"], f32) + inputs["b_ctx"]
    mu = ctx.mean(-1, keepdims=True)
    var = ctx.var(-1, keepdims=True)
    ctx = (ctx - mu) / np.sqrt(var + EPS) * inputs["ln_g"] + inputs["ln_b"]
    x = x + ctx
    q = (x @ np.asarray(inputs["wq"], f32) + inputs["bq"]).reshape(Bf, Nf, H, DK).transpose(0, 2, 1, 3)
    k = (x @ np.asarray(inputs["wk"], f32) + inputs["bk"]).reshape(Bf, Nf, H, DK).transpose(0, 2, 1, 3)
    v = (x @ np.asarray(inputs["wv"], f32) + inputs["bv"]).reshape(Bf, Nf, H, DK).transpose(0, 2, 1, 3)
    sc = np.einsum('bhid,bhjd->bhij', q, k) / np.sqrt(DK)
    e2 = np.exp(sc - sc.max(-1, keepdims=True))
    att = e2 / e2.sum(-1, keepdims=True)
    out = np.einsum('bhij,bhjd->bhid', att, v).transpose(0, 2, 1, 3).reshape(Bf, Nf, Df)
    out = out @ np.asarray(inputs["wo"], f32) + inputs["bo"]
    imp = att.mean(axis=1).mean(axis=1)
    gwv = np.exp(imp - imp.max(-1, keepdims=True))
    gwv = gwv / gwv.sum(-1, keepdims=True)
    agg = (out * gwv[..., None]).sum(axis=1)
    s = np.asarray(inputs["bn_g"], f32) / np.sqrt(np.asarray(inputs["bn_v"], f32) + EPS)
    y = relu((agg @ np.asarray(inputs["wc1"], f32) + inputs["bc1"]
              - inputs["bn_m"]) * s + inputs["bn_b"])
    return (y @ np.asarray(inputs["wc2"], f32) + inputs["bc2"]).astype(f32)


def kernel(**inputs):
    try:
        nc = get_nc()
        shared = host_inputs(inputs)
        in_maps = [core_input(inputs, shared, c) for c in range(NCORES)]
        res = run_bass_kernel_spmd(nc, in_maps, core_ids=list(range(NCORES)))
        out = np.concatenate(
            [np.ascontiguousarray(res.results[c]["outT"].T) for c in range(NCORES)],
            axis=0)
        return out.astype(np.float32)
    except Exception:
        import traceback
        traceback.print_exc()
        return _reference_numpy(inputs)

